# revision 17
# baseline (speedup 1.0000x reference)
"""Distributed RGCN+GraphConv (gated residual) kernel for 8 Trainium2 cores.

Sharding: target nodes are bin-packed into bins of <=16 nodes whose total
in-degree is <=256.  Each core owns BINS_C consecutive bins (graph/data
parallel over targets).  Edge lists are padded per-bin to a uniform structure
so a single SPMD NEFF serves all cores.  Messages are aggregated
feature-major via one-hot scatter-matmuls on the tensor engine; the
relation-weight product is applied after aggregation (A-then-W).  h1 is
exchanged via AllGather.

v2: edge gathers round-robin over 4 SWDGE queues (a queue's descriptor ring
serializes a gather with the previous transfer on the same queue); one-hot
scatter matrices are prebuilt on the host and streamed from HBM; residual/
gate arithmetic in bf16.
v3: fp8(e4m3) node/h1 tables and scatter matrices (halves gather DMA and
S-stream bytes); scatter matmuls use fp8 DoubleRow perf mode (256-edge
contraction per instruction); AllGather pipelined per tile group.
"""

import numpy as np
import ml_dtypes

import concourse.bacc as bacc
import concourse.mybir as mybir
import concourse.tile as tile
from concourse.library_config import mlp as _mlp_lib
from concourse.bass_utils import run_bass_kernel_spmd

BF16 = ml_dtypes.bfloat16
FP8 = ml_dtypes.float8_e4m3

N = 20000
E = 320000
R = 8
G = 256          # feature width (g_dim == h1_dim == h2_dim)
CORES = 8
P = 128
SLOT = 16        # target slots per bin
CAP = 256        # edge slots per bin (2 chunks of 128)
TG = 512         # targets per tile-group
NQ = 4           # SWDGE queues for edge gathers

F32 = mybir.dt.float32
BF = mybir.dt.bfloat16
F8 = mybir.dt.float8e4
I16 = mybir.dt.int16
DR = mybir.MatmulPerfMode.DoubleRow

_nc_cache: dict = {}


# ----------------------------------------------------------------------------
# host-side: bin packing of target nodes
# ----------------------------------------------------------------------------

def _pack_bins(deg: np.ndarray, bins_c: int):
    """LPT pack nodes into CORES*bins_c bins (<=SLOT nodes, <=CAP edge sum).

    Returns (bin_of_node, slot_in_bin) or None if infeasible."""
    import heapq

    nbins = CORES * bins_c
    order = np.argsort(-deg, kind="stable")
    heap = [(0, b) for b in range(nbins)]
    heapq.heapify(heap)
    counts = np.zeros(nbins, np.int32)
    sums = np.zeros(nbins, np.int64)
    bin_of = np.full(N, -1, np.int32)
    slot_of = np.full(N, -1, np.int32)
    stash = []
    for n in order:
        d = int(deg[n])
        placed = False
        while heap:
            s, b = heapq.heappop(heap)
            if counts[b] >= SLOT:
                continue        # bin full by count; drop from heap
            if s + d > CAP:
                stash.append((s, b))
                # smallest-sum bin can't take it -> no bin can (heap is by sum)
                break
            bin_of[n] = b
            slot_of[n] = counts[b]
            counts[b] += 1
            sums[b] = s + d
            if counts[b] < SLOT:
                heapq.heappush(heap, (int(sums[b]), b))
            placed = True
            break
        for item in stash:
            heapq.heappush(heap, item)
        stash.clear()
        if not placed:
            return None
    return bin_of, slot_of


# ----------------------------------------------------------------------------
# device kernel builder (structure depends only on bins_c)
# ----------------------------------------------------------------------------

def _build_nc(bins_c: int):
    t_c = bins_c * SLOT              # targets per core
    npad = bins_c * CAP              # edge slots per core
    ntg = t_c // TG                  # tile groups
    nidxcol = npad // 16
    sctg = TG // SLOT * CAP // P     # S chunk-columns per tile group (=64)

    nc = bacc.Bacc("TRN2", target_bir_lowering=False, debug=False,
                   num_devices=CORES, num_swdge_queues=NQ)

    t_xtab = nc.dram_tensor("x_tab", [N, G], F8, kind="ExternalInput")
    t_xT = nc.dram_tensor("xT", [P, 2, t_c], BF, kind="ExternalInput")
    t_idx1 = nc.dram_tensor("idx1", [P, nidxcol], I16, kind="ExternalInput")
    t_idx2 = nc.dram_tensor("idx2", [P, nidxcol], I16, kind="ExternalInput")
    t_s1 = nc.dram_tensor("s1", [P, npad // P, P], F8, kind="ExternalInput")
    t_s2 = nc.dram_tensor("s2", [P, npad // P, P], BF, kind="ExternalInput")
    t_wfull = nc.dram_tensor("wfull", [P, 16, G], BF, kind="ExternalInput")
    t_root1 = nc.dram_tensor("root1", [P, 2, G], BF, kind="ExternalInput")
    t_g1w = nc.dram_tensor("g1w", [P, 4, G], BF, kind="ExternalInput")
    t_wrel = nc.dram_tensor("wrel", [P, 2, G], BF, kind="ExternalInput")
    t_wroot = nc.dram_tensor("wroot", [P, 2, G], BF, kind="ExternalInput")
    t_g2w = nc.dram_tensor("g2w", [P, 4, G], BF, kind="ExternalInput")
    t_bias = nc.dram_tensor("biases", [P, 8], F32, kind="ExternalInput")
    t_ident = nc.dram_tensor("ident", [P, P], BF, kind="ExternalInput")

    t_out = nc.dram_tensor("h2T", [2, P, t_c], BF, kind="ExternalOutput")

    d_h1own = nc.dram_tensor("h1_own", [t_c, G], BF)
    # table layout: [tb][core][TG] rows so per-tile-group AllGathers land
    # in contiguous blocks
    d_h1tab = nc.dram_tensor("h1_tab", [CORES * t_c, G], BF,
                             addr_space="Shared")

    Iden = mybir.ActivationFunctionType.Identity
    Sigm = mybir.ActivationFunctionType.Sigmoid
    MUL = mybir.AluOpType.mult
    SUB = mybir.AluOpType.subtract
    ADD = mybir.AluOpType.add

    with tile.TileContext(nc, num_cores=CORES) as tc:
        with tc.tile_pool(name="cst", bufs=1) as cst, \
             tc.tile_pool(name="res", bufs=1) as res, \
             tc.tile_pool(name="pA", bufs=2) as pA, \
             tc.tile_pool(name="pG", bufs=2) as pG, \
             tc.tile_pool(name="pGf", bufs=4) as pGf, \
             tc.tile_pool(name="pS", bufs=2) as pS, \
             tc.tile_pool(name="pH", bufs=2) as pH, \
             tc.tile_pool(name="psA", bufs=2, space="PSUM") as psA, \
             tc.tile_pool(name="psD", bufs=2, space="PSUM") as psD, \
             tc.tile_pool(name="psT", bufs=2, space="PSUM") as psT:

            nc.gpsimd.load_library(_mlp_lib)

            # ------- load constants / weights -------
            idx1_t = cst.tile([P, nidxcol], I16)
            nc.sync.dma_start(out=idx1_t[:], in_=t_idx1[:])
            idx2_t = cst.tile([P, nidxcol], I16)
            nc.sync.dma_start(out=idx2_t[:], in_=t_idx2[:])
            wfull_t = cst.tile([P, 16, G], BF)
            nc.sync.dma_start(out=wfull_t[:], in_=t_wfull[:])
            root1_t = cst.tile([P, 2, G], BF)
            nc.sync.dma_start(out=root1_t[:], in_=t_root1[:])
            g1w_t = cst.tile([P, 4, G], BF)
            nc.sync.dma_start(out=g1w_t[:], in_=t_g1w[:])
            wrel_t = cst.tile([P, 2, G], BF)
            nc.sync.dma_start(out=wrel_t[:], in_=t_wrel[:])
            wroot_t = cst.tile([P, 2, G], BF)
            nc.sync.dma_start(out=wroot_t[:], in_=t_wroot[:])
            g2w_t = cst.tile([P, 4, G], BF)
            nc.sync.dma_start(out=g2w_t[:], in_=t_g2w[:])
            bias_t = cst.tile([P, 8], F32)
            nc.sync.dma_start(out=bias_t[:], in_=t_bias[:])
            ident_t = cst.tile([P, P], BF)
            nc.sync.dma_start(out=ident_t[:], in_=t_ident[:])

            # ------- resident node-feature tiles (feature-major, bf16) -------
            xT_b = res.tile([P, 2, t_c], BF)
            nc.sync.dma_start(out=xT_b[:], in_=t_xT[:])
            h1T_b = res.tile([P, 2, t_c], BF)

            # ================= Layer 1 =================
            # one-stage software pipeline: scatter(tb) is emitted before
            # dense/gate/transpose(tb-1) so PE never idles waiting for the
            # A_bf copies (idle resets the PE p-state ramp)
            A_tiles = [None] * ntg

            def l1_front(tb):
                s1t = pS.tile([P, sctg, P], F8, tag="s1")
                nc.scalar.dma_start(out=s1t[:],
                                    in_=t_s1[:, tb * sctg:(tb + 1) * sctg, :])
                A_bf = pA.tile([P, 2, TG * 8], BF, tag="A")
                A_tiles[tb] = A_bf
                for bank in range(8):
                    ccol = tb * 512 + bank * 64
                    xg = pGf.tile([P, 8, G], F8, tag=f"f{bank % NQ}")
                    nc.gpsimd.dma_gather(
                        out_ap=xg[:], in_ap=t_xtab[:],
                        idxs_ap=idx1_t[:, ccol:ccol + 64],
                        num_idxs=1024, num_idxs_reg=1024, elem_size=G,
                        queue_num=bank % NQ)
                    aps = [psA.tile([P, 512], F32, tag=f"psA{g}",
                                    name=f"apsL1_{tb}_{bank}_{g}")
                           for g in range(2)]
                    for cc in range(8):
                        col = bank * 8 + cc
                        b4 = cc // 2
                        for gh in range(2):
                            nc.tensor.matmul(
                                out=aps[gh][:].rearrange(
                                    "p (r x) -> p r x",
                                    r=R)[:, :, b4 * 16:(b4 + 1) * 16],
                                lhsT=xg[:, cc, gh * P:(gh + 1) * P],
                                rhs=s1t[:, col, :],
                                start=(cc == 0), stop=(cc == 7))
                    nc.scalar.copy(
                        out=A_bf[:, 0, bank * 512:(bank + 1) * 512],
                        in_=aps[0][:])
                    nc.vector.tensor_copy(
                        out=A_bf[:, 1, bank * 512:(bank + 1) * 512],
                        in_=aps[1][:])

            def l1_back(tb):
                A_bf = A_tiles[tb]
                sl = slice(tb * TG, (tb + 1) * TG)
                h1g_b = pH.tile([P, 2, TG], BF, tag="h1g_b")
                for hh in range(2):
                    agg = psD.tile([P, TG], F32, tag="agg",
                                   name=f"aggL1_{tb}_{hh}")
                    for gh in range(2):
                        nc.tensor.matmul(
                            out=agg[:],
                            lhsT=root1_t[:, gh, hh * P:(hh + 1) * P],
                            rhs=xT_b[:, gh, sl],
                            start=(gh == 0), stop=False)
                    k = 0
                    for r in range(R):
                        for gh in range(2):
                            k += 1
                            nc.tensor.matmul(
                                out=agg[:],
                                lhsT=wfull_t[:, r * 2 + gh,
                                             hh * P:(hh + 1) * P],
                                rhs=A_bf[:, gh].rearrange(
                                    "p (b r t) -> p r b t",
                                    b=8, r=R, t=64)[:, r],
                                start=False, stop=(k == 16))
                    nc.scalar.activation(out=h1g_b[:, hh], in_=agg[:],
                                         func=Iden, bias=bias_t[:, 0 + hh:1 + hh])
                for hh in range(2):
                    gps = psD.tile([P, TG], F32, tag="agg",
                                   name=f"gpsL1_{tb}_{hh}")
                    rhs4 = [xT_b[:, 0, sl], xT_b[:, 1, sl],
                            h1g_b[:, 0], h1g_b[:, 1]]
                    for k4 in range(4):
                        nc.tensor.matmul(
                            out=gps[:],
                            lhsT=g1w_t[:, k4, hh * P:(hh + 1) * P],
                            rhs=rhs4[k4],
                            start=(k4 == 0), stop=(k4 == 3))
                    alpha = pH.tile([P, TG], BF, tag="alpha")
                    nc.scalar.activation(out=alpha[:], in_=gps[:],
                                         func=Sigm, bias=bias_t[:, 2 + hh:3 + hh])
                    d = pH.tile([P, TG], BF, tag="d")
                    nc.vector.tensor_tensor(out=d[:], in0=h1g_b[:, hh],
                                            in1=xT_b[:, hh, sl], op=SUB)
                    m = pH.tile([P, TG], BF, tag="m")
                    nc.vector.tensor_tensor(out=m[:], in0=alpha[:], in1=d[:],
                                            op=MUL)
                    nc.vector.tensor_tensor(out=h1T_b[:, hh, sl], in0=m[:],
                                            in1=xT_b[:, hh, sl], op=ADD)
                for j in range(TG // P):
                    own = pH.tile([P, G], BF, tag="own")
                    for hh in range(2):
                        tp = psT.tile([P, P], BF, tag="tp",
                                      name=f"tp_{tb}_{j}_{hh}")
                        nc.tensor.transpose(
                            out=tp[:],
                            in_=h1T_b[:, hh, tb * TG + j * P:tb * TG + (j + 1) * P],
                            identity=ident_t[:])
                        nc.scalar.copy(out=own[:, hh * P:(hh + 1) * P],
                                       in_=tp[:])
                    rr = tb * TG + j * P
                    nc.sync.dma_start(out=d_h1own[rr:rr + P, :], in_=own[:])

            for tb in range(ntg + 1):
                if tb < ntg:
                    l1_front(tb)
                if tb >= 1:
                    l1_back(tb - 1)
                if tb == 3:        # groups 0-2 done
                    nc.gpsimd.collective_compute(
                        "AllGather", mybir.AluOpType.bypass,
                        replica_groups=[list(range(CORES))],
                        ins=[d_h1own[0:3 * TG].opt()],
                        outs=[d_h1tab[0:3 * TG * CORES].opt()])
                if tb == ntg:      # groups 3-4 done
                    nc.gpsimd.collective_compute(
                        "AllGather", mybir.AluOpType.bypass,
                        replica_groups=[list(range(CORES))],
                        ins=[d_h1own[3 * TG:ntg * TG].opt()],
                        outs=[d_h1tab[3 * TG * CORES:ntg * TG * CORES].opt()])

            # ================= Layer 2 =================
            A2_tiles = [None] * ntg

            def l2_front(tb):
                s2t = pS.tile([P, sctg // 2, P], BF, tag="s2")
                nc.scalar.dma_start(
                    out=s2t[:],
                    in_=t_s2[:, tb * sctg:tb * sctg + sctg // 2, :])
                s2u = pS.tile([P, sctg // 2, P], BF, tag="s2")
                nc.scalar.dma_start(
                    out=s2u[:],
                    in_=t_s2[:, tb * sctg + sctg // 2:(tb + 1) * sctg, :])
                a2ps = [psA.tile([P, TG], F32, tag=f"psA{g}",
                                 name=f"apsL2_{tb}_{g}") for g in range(2)]
                for call in range(8):
                    ccol = tb * 512 + call * 64
                    blk = call // 2
                    hg = pG.tile([P, 8, G], BF, tag=f"g{call % NQ}")
                    nc.gpsimd.dma_gather(
                        out_ap=hg[:], in_ap=d_h1tab[:],
                        idxs_ap=idx2_t[:, ccol:ccol + 64],
                        num_idxs=1024, num_idxs_reg=1024, elem_size=G,
                        queue_num=call % NQ)
                    for cc in range(8):
                        col = call * 8 + cc
                        st = s2t if col < sctg // 2 else s2u
                        for gh in range(2):
                            nc.tensor.matmul(
                                out=a2ps[gh][:, blk * P:(blk + 1) * P],
                                lhsT=hg[:, cc, gh * P:(gh + 1) * P],
                                rhs=st[:, col % (sctg // 2), :],
                                start=(call == 0 and cc == 0),
                                stop=(call == 7 and cc == 7))
                A2_bf = pH.tile([P, 2, TG], BF, tag="A2")
                A2_tiles[tb] = A2_bf
                nc.scalar.copy(out=A2_bf[:, 0], in_=a2ps[0][:])
                nc.vector.tensor_copy(out=A2_bf[:, 1], in_=a2ps[1][:])

            def l2_back(tb):
                A2_bf = A2_tiles[tb]
                sl = slice(tb * TG, (tb + 1) * TG)
                h2g_b = pH.tile([P, 2, TG], BF, tag="h1g_b")
                for hh in range(2):
                    agg = psD.tile([P, TG], F32, tag="agg",
                                   name=f"aggL2_{tb}_{hh}")
                    for gh in range(2):
                        nc.tensor.matmul(
                            out=agg[:],
                            lhsT=wroot_t[:, gh, hh * P:(hh + 1) * P],
                            rhs=h1T_b[:, gh, sl],
                            start=(gh == 0), stop=False)
                    for gh in range(2):
                        nc.tensor.matmul(
                            out=agg[:],
                            lhsT=wrel_t[:, gh, hh * P:(hh + 1) * P],
                            rhs=A2_bf[:, gh],
                            start=False, stop=(gh == 1))
                    nc.scalar.activation(out=h2g_b[:, hh], in_=agg[:],
                                         func=Iden, bias=bias_t[:, 4 + hh:5 + hh])
                for hh in range(2):
                    gps = psD.tile([P, TG], F32, tag="agg",
                                   name=f"gpsL2_{tb}_{hh}")
                    rhs4 = [h1T_b[:, 0, sl], h1T_b[:, 1, sl],
                            h2g_b[:, 0], h2g_b[:, 1]]
                    for k4 in range(4):
                        nc.tensor.matmul(
                            out=gps[:],
                            lhsT=g2w_t[:, k4, hh * P:(hh + 1) * P],
                            rhs=rhs4[k4],
                            start=(k4 == 0), stop=(k4 == 3))
                    alpha = pH.tile([P, TG], BF, tag="alpha")
                    nc.scalar.activation(out=alpha[:], in_=gps[:],
                                         func=Sigm, bias=bias_t[:, 6 + hh:7 + hh])
                    d = pH.tile([P, TG], BF, tag="d")
                    nc.vector.tensor_tensor(out=d[:], in0=h2g_b[:, hh],
                                            in1=h1T_b[:, hh, sl], op=SUB)
                    m = pH.tile([P, TG], BF, tag="m")
                    nc.vector.tensor_tensor(out=m[:], in0=alpha[:], in1=d[:],
                                            op=MUL)
                    h2 = pH.tile([P, TG], BF, tag="h2")
                    nc.vector.tensor_tensor(out=h2[:], in0=m[:],
                                            in1=h1T_b[:, hh, sl], op=ADD)
                    nc.sync.dma_start(out=t_out[hh, :, sl], in_=h2[:])

            for tb in range(ntg + 1):
                if tb < ntg:
                    l2_front(tb)
                if tb >= 1:
                    l2_back(tb - 1)

    nc.compile()
    return nc


# ----------------------------------------------------------------------------
# host-side preprocessing + launch
# ----------------------------------------------------------------------------

def _wrap_idx(idx_pad: np.ndarray) -> np.ndarray:
    """[npad] int16 -> [128, npad/16] wrapped (i at [i%16, i//16]) + replicated."""
    w = idx_pad.reshape(-1, 16).T
    return np.ascontiguousarray(np.tile(w, (8, 1)))


def prepare(inputs: dict):
    node_features = np.asarray(inputs["node_features"], np.float32)
    edge_index = np.asarray(inputs["edge_index"], np.int64)
    edge_norm = np.asarray(inputs["edge_norm"], np.float32)
    edge_type = np.asarray(inputs["edge_type"], np.int64)
    basis = np.asarray(inputs["basis"], np.float32)
    comp = np.asarray(inputs["comp"], np.float32)
    root1 = np.asarray(inputs["root1"], np.float32)
    bias1 = np.asarray(inputs["bias1"], np.float32)
    w_rel = np.asarray(inputs["w_rel"], np.float32)
    b_rel = np.asarray(inputs["b_rel"], np.float32)
    w_root = np.asarray(inputs["w_root"], np.float32)
    gate1_w = np.asarray(inputs["gate1_w"], np.float32)
    gate1_b = np.asarray(inputs["gate1_b"], np.float32)
    gate2_w = np.asarray(inputs["gate2_w"], np.float32)
    gate2_b = np.asarray(inputs["gate2_b"], np.float32)

    src = edge_index[0].astype(np.int64)
    tgt = edge_index[1].astype(np.int64)
    rel = edge_type.astype(np.int64)

    deg = np.bincount(tgt, minlength=N)
    bins_c = -(-max(N // SLOT + 1, (E + CORES * CAP - 1) // (CORES * CAP)) // (CORES * 32)) * 32
    bins_c = max(bins_c, 32)
    packed = None
    while packed is None:
        packed = _pack_bins(deg, bins_c)
        if packed is None:
            bins_c += 32
            if bins_c > 224:
                raise RuntimeError("bin packing failed")
    bin_of, slot_of = packed
    t_c = bins_c * SLOT
    npad = bins_c * CAP
    ntg = t_c // TG

    core_of = bin_of // bins_c
    bin_loc = bin_of % bins_c
    tslot_of = bin_loc * SLOT + slot_of          # target slot within core
    # h1 table row: two AllGather blocks [0:3TG) and [3TG:5TG), each
    # laid out [core][rows-in-block]
    table_pos = np.where(
        tslot_of < 3 * TG,
        core_of * (3 * TG) + tslot_of,
        3 * TG * CORES + core_of * (2 * TG) + (tslot_of - 3 * TG))

    # per-relation mean normalization (computed from the ORIGINAL graph)
    segid = tgt * R + rel
    cnt = np.bincount(segid, minlength=N * R).astype(np.float64)
    scale_e = (1.0 / np.maximum(cnt, 1.0))[segid].astype(np.float32)

    # global edge ordering: (core, bin_loc, slot_of_tgt, rel)
    ek = np.lexsort((rel, slot_of[tgt], bin_loc[tgt], core_of[tgt]))
    e_core = core_of[tgt][ek]
    e_bin = bin_loc[tgt][ek]

    # position of each edge inside its core's padded slot array
    key = e_core.astype(np.int64) * bins_c + e_bin
    uniq, inv, counts = np.unique(key, return_inverse=True, return_counts=True)
    start = np.zeros(len(uniq), np.int64)
    np.cumsum(counts[:-1], out=start[1:])
    offs = np.arange(len(key)) - start[inv]
    if counts.max() > CAP:
        raise RuntimeError("bin overflow")
    slot_idx = e_bin * CAP + offs                 # edge slot within core

    # build per-core streams
    w_full = np.einsum("rb,bio->rio", comp, basis).astype(np.float32)
    wfull_pack = np.ascontiguousarray(
        w_full.reshape(R, 2, P, G).transpose(2, 0, 1, 3).reshape(P, 16, G)
    ).astype(BF16)
    root1_pack = np.ascontiguousarray(
        root1.reshape(2, P, G).transpose(1, 0, 2)).astype(BF16)
    g1w_pack = np.ascontiguousarray(
        gate1_w.reshape(4, P, G).transpose(1, 0, 2)).astype(BF16)
    wrel_pack = np.ascontiguousarray(
        w_rel.reshape(2, P, G).transpose(1, 0, 2)).astype(BF16)
    wroot_pack = np.ascontiguousarray(
        w_root.reshape(2, P, G).transpose(1, 0, 2)).astype(BF16)
    g2w_pack = np.ascontiguousarray(
        gate2_w.reshape(4, P, G).transpose(1, 0, 2)).astype(BF16)
    bias_pack = np.stack([bias1.reshape(2, P), gate1_b.reshape(2, P),
                          b_rel.reshape(2, P), gate2_b.reshape(2, P)], 0)
    bias_pack = np.ascontiguousarray(
        bias_pack.reshape(8, P).T).astype(np.float32)   # [128, 8]
    ident = np.eye(P, dtype=np.float32).astype(BF16)
    x_bf = node_features.astype(FP8)

    in_maps = []
    for c in range(CORES):
        mask = e_core == c
        sl = slot_idx[mask]
        eidx = ek[mask]
        idx1 = np.zeros(npad, np.int16)
        idx1[sl] = src[eidx].astype(np.int16)
        idx2 = np.zeros(npad, np.int16)
        idx2[sl] = table_pos[src[eidx]].astype(np.int16)

        # streamed one-hot*scale scatter matrices [128, npad/128, 128]
        # edge at slot e: partition e%128, chunk e//128;
        # L1 column within chunk = slot_of(tgt)*R + rel
        # L2 column within chunk = (bin_loc%8)*SLOT + slot_of(tgt)
        s1 = np.zeros((P, npad // P, P), np.float32)
        s2 = np.zeros((P, npad // P, P), np.float32)
        ecol1 = (rel[eidx] * SLOT + slot_of[tgt[eidx]]).astype(np.int64)
        ecol2 = ((bin_loc[tgt[eidx]] % 8) * SLOT
                 + slot_of[tgt[eidx]]).astype(np.int64)
        s1[sl % P, sl // P, ecol1] = scale_e[eidx]
        s2[sl % P, sl // P, ecol2] = edge_norm[eidx]

        # x of this core's targets, feature-major [128, 2, t_c] bf16
        nodes_c = np.where(core_of == c)[0]
        xTc = np.zeros((t_c, G), np.float32)
        xTc[tslot_of[nodes_c]] = node_features[nodes_c]
        xT_pack = np.ascontiguousarray(
            xTc.T.reshape(2, P, t_c).transpose(1, 0, 2)).astype(BF16)

        in_maps.append({
            "x_tab": x_bf,
            "xT": xT_pack,
            "idx1": _wrap_idx(idx1),
            "idx2": _wrap_idx(idx2),
            "s1": s1.astype(FP8),
            "s2": s2.astype(BF16),
            "wfull": wfull_pack,
            "root1": root1_pack,
            "g1w": g1w_pack,
            "wrel": wrel_pack,
            "wroot": wroot_pack,
            "g2w": g2w_pack,
            "biases": bias_pack,
            "ident": ident,
        })

    meta = (bins_c, core_of, tslot_of)
    return in_maps, meta


def postprocess(results, meta):
    bins_c, core_of, tslot_of = meta
    t_c = bins_c * SLOT
    out = np.empty((N, G), np.float32)
    for c in range(CORES):
        h2T = np.asarray(results[c]["h2T"]).astype(np.float32)  # [2,128,t_c]
        h2 = h2T.reshape(G, t_c).T               # [t_c, 256]
        nodes_c = np.where(core_of == c)[0]
        out[nodes_c] = h2[tslot_of[nodes_c]]
    return out


def run(inputs: dict, trace: bool = False):
    import time as _time
    in_maps, meta = prepare(inputs)
    bins_c = meta[0]
    if bins_c not in _nc_cache:
        _t = _time.time()
        _nc_cache[bins_c] = _build_nc(bins_c)
        print(f"[kernel] build+compile {_time.time() - _t:.1f}s", flush=True)
    nc = _nc_cache[bins_c]
    _t = _time.time()
    res = run_bass_kernel_spmd(nc, in_maps, core_ids=list(range(CORES)),
                               trace=trace)
    print(f"[kernel] exec {_time.time() - _t:.1f}s", flush=True)
    out = postprocess(res.results, meta)
    return out, res


def kernel(**inputs) -> np.ndarray:
    out, _ = run(inputs, trace=False)
    return out
